# revision 2
# baseline (speedup 1.0000x reference)
"""Trainium2 Bass kernel for DTW features (open-end weighted DTW).

Problem: x (64, 6, 2048) f32, patts (64, 32) f32, w scalar.
  c[i,j]   = (patts[n,i] - x[b,d,j])^2
  D[0,j]   = c[0,j]
  D[i,j]   = c[i,j] + w * min(D[i-1,j], D[i,j-1], D[i-1,j-1])
  out[b,n,d,j] = sqrt(D[L-1,j])

Strategy: data-parallel over batch (8 batches per core).  Per (b, n, d)
tuple the DP runs row-by-row in the scaled domain Dt[i,j] = D[i,j]*w^-j,
which turns the recurrence into a hardware min/add scan along j:

  ct[i,j]  = c[i,j] * w^-j
  u[j]     = min(w*Dt[i-1,j], Dt[i-1,j-1])          (scalar_tensor_tensor)
  Dt[i,j]  = min(u[j], Dt[i,j-1]) + ct[i,j]         (tensor_tensor_scan)
  out      = sqrt(Dt[L-1,j] * w^j)

The cost matrix ct is produced by the TensorEngine as a rank-6 matmul:
ct[(s,n), j] = [p^2, -2p, 1] . [w^-j, x_s*w^-j, x_s^2*w^-j] with two
sequences s packed per 128-partition block (64 patterns each half).
"""

import os
import sys

import numpy as np

for _p in ("/opt/trn_rl_repo", "/root/.axon_site/_ro/trn_rl_repo"):
    if _p not in sys.path and os.path.isdir(_p):
        sys.path.insert(0, _p)

B, N, D, L, T = 64, 64, 6, 32, 2048
NCORES = 8
BLOC = B // NCORES            # batches per core
NSEQ = BLOC * D               # (b, d) sequences per core
NBLK = NSEQ // 2              # two sequences per 128-partition block
P, HALF = 128, 64
BIG = 1.0e30

_cache = {}


def _build(nblk, l_patts, t_len, w):
    """Build + compile the per-core Bass program (SPMD across 8 cores)."""
    import concourse.bacc as bacc
    import concourse.bass as bass
    import concourse.mybir as mybir
    import concourse.tile as tile

    f32 = mybir.dt.float32
    Alu = mybir.AluOpType
    Act = mybir.ActivationFunctionType
    CHUNK = min(512, t_len)
    nchunk = t_len // CHUNK

    nc = bacc.Bacc("TRN2", target_bir_lowering=False, debug=False,
                   num_devices=NCORES)

    rhs_d = nc.dram_tensor("rhs", [nblk, 6, t_len], f32, kind="ExternalInput")
    lhsT_d = nc.dram_tensor("lhsT", [6, l_patts * P], f32, kind="ExternalInput")
    wj_d = nc.dram_tensor("wj", [P, t_len], f32, kind="ExternalInput")
    out_d = nc.dram_tensor("out", [nblk, P, t_len], f32, kind="ExternalOutput")

    with tile.TileContext(nc) as tc:
        with (
            tc.tile_pool(name="const", bufs=1) as cpool,
            tc.tile_pool(name="rhs", bufs=2) as rpool,
            tc.tile_pool(name="rows", bufs=2) as dpool,
            tc.tile_pool(name="work", bufs=2) as wpool,
            tc.tile_pool(name="outp", bufs=2) as opool,
            tc.tile_pool(name="psum", bufs=2, space=bass.MemorySpace.PSUM) as ppool,
        ):
            lhsT_sb = cpool.tile([6, l_patts * P], f32)
            nc.sync.dma_start(lhsT_sb[:], lhsT_d[:])
            wj_sb = cpool.tile([P, t_len], f32)
            nc.sync.dma_start(wj_sb[:], wj_d[:])

            for blk in range(nblk):
                rhs_sb = rpool.tile([6, t_len], f32, tag="rhs")
                nc.sync.dma_start(rhs_sb[:], rhs_d[blk])

                # ping-pong row-state tiles; col 0 is the j=-1 boundary
                dA = dpool.tile([P, t_len + 1], f32, tag="dA")
                dB = dpool.tile([P, t_len + 1], f32, tag="dB")
                nc.gpsimd.memset(dA[:, 0:1], BIG)
                nc.gpsimd.memset(dB[:, 0:1], BIG)
                rows = [dA, dB]

                for i in range(l_patts):
                    ct = ppool.tile([P, t_len], f32, tag="ct")
                    for k in range(nchunk):
                        nc.tensor.matmul(
                            ct[:, k * CHUNK:(k + 1) * CHUNK],
                            lhsT_sb[:, i * P:(i + 1) * P],
                            rhs_sb[:, k * CHUNK:(k + 1) * CHUNK],
                            start=True, stop=True,
                        )
                    cur = rows[i % 2]
                    if i == 0:
                        # row 0: Dt = ct
                        nc.scalar.activation(cur[:, 1:t_len + 1], ct[:], Act.Copy)
                    else:
                        prev = rows[(i - 1) % 2]
                        u = wpool.tile([P, t_len], f32, tag="u")
                        nc.vector.scalar_tensor_tensor(
                            u[:], prev[:, 1:t_len + 1], w, prev[:, 0:t_len],
                            Alu.mult, Alu.min,
                        )
                        nc.vector.tensor_tensor_scan(
                            cur[:, 1:t_len + 1], u[:], ct[:], BIG,
                            Alu.min, Alu.add,
                        )

                last = rows[(l_patts - 1) % 2]
                sq = wpool.tile([P, t_len], f32, tag="sq")
                # clamp tiny negative fp noise, then unscale by w^j
                nc.vector.scalar_tensor_tensor(
                    sq[:], last[:, 1:t_len + 1], 0.0, wj_sb[:],
                    Alu.max, Alu.mult,
                )
                ot = opool.tile([P, t_len], f32, tag="ot")
                nc.scalar.activation(ot[:], sq[:], Act.Sqrt)
                nc.sync.dma_start(out_d[blk], ot[:])

    nc.compile()
    return nc


def _host_prep(x, patts, w):
    """Per-core input arrays for the SPMD kernel."""
    wf = np.float64(np.float32(w))
    invw = (wf ** -np.arange(T)).astype(np.float32)          # w^-j
    wj = (wf ** np.arange(T)).astype(np.float32)             # w^j
    wj_bcast = np.broadcast_to(wj, (P, T)).copy()

    p = np.asarray(patts, np.float32)                        # (N, L)
    lhsT = np.zeros((6, L, P), np.float32)
    for i in range(L):
        pi = p[:, i]
        lhsT[0, i, :HALF] = pi * pi
        lhsT[1, i, :HALF] = -2.0 * pi
        lhsT[2, i, :HALF] = 1.0
        lhsT[3, i, HALF:] = pi * pi
        lhsT[4, i, HALF:] = -2.0 * pi
        lhsT[5, i, HALF:] = 1.0
    lhsT = lhsT.reshape(6, L * P)

    xf = np.asarray(x, np.float32)
    in_maps = []
    for c in range(NCORES):
        xs = xf[c * BLOC:(c + 1) * BLOC].reshape(NSEQ, T)    # (48, 2048)
        r1 = (xs * invw[None, :]).astype(np.float32)
        r2 = (xs * xs * invw[None, :]).astype(np.float32)
        rhs = np.empty((NBLK, 6, T), np.float32)
        rhs[:, 0] = invw
        rhs[:, 1] = r1[0::2]
        rhs[:, 2] = r2[0::2]
        rhs[:, 3] = invw
        rhs[:, 4] = r1[1::2]
        rhs[:, 5] = r2[1::2]
        in_maps.append({"rhs": rhs, "lhsT": lhsT, "wj": wj_bcast})
    return in_maps


def kernel(x, patts, w):
    from concourse.bass_utils import run_bass_kernel_spmd

    wv = float(np.float32(w))
    key = ("prog", NBLK, L, T, wv)
    if key not in _cache:
        _cache[key] = _build(NBLK, L, T, wv)
    nc = _cache[key]

    in_maps = _host_prep(x, patts, w)
    trace = bool(int(os.environ.get("DTW_TRACE", "0")))
    res = run_bass_kernel_spmd(nc, in_maps, list(range(NCORES)), trace=trace)
    _cache["last_results"] = res

    outs = []
    for c in range(NCORES):
        o = res.results[c]["out"]                            # (NBLK, 128, T)
        o = o.reshape(NBLK, 2, N, T).reshape(NSEQ, N, T)     # seq-major
        o = o.reshape(BLOC, D, N, T).transpose(0, 2, 1, 3)   # (b, n, d, t)
        outs.append(o)
    return np.ascontiguousarray(np.concatenate(outs, axis=0).astype(np.float32))
